# revision 16
# baseline (speedup 1.0000x reference)
"""DepthRelationEmbedding Trainium2 kernel (v4).

Math: out[h,n,hw] = relu( sum_d pos[n,hw,d] * W[d,h] + b[h] ) where pos is the
interleaved sin/cos embedding of delta[n,hw] = ln((relu(pd[n])+eps)/(dm[hw]+eps)).

Angle addition (We = W[0::2], Wo = W[1::2]):
  out[(n,h), hw] = sum_k U[k,(n,h)]*cosC[k,hw] + V[k,(n,h)]*sinC[k,hw]
  U = sinA*We + cosA*Wo,  V = sinA*Wo - cosA*We
so the (N,HW,256) intermediate never exists; per core the output is one
[608 x 256] @ [256 x 960] bf16 matmul pair accumulated in PSUM.

Angles in turns (tau = angle/2pi); range reduction f = tau - rint(tau) via the
fp32->int32 copy (rounds-to-nearest on HW); sin = Sin(2pi f),
cos = Sin(pi/2 - 2pi|f|).

v4 changes vs v3:
 - PE warmups source from memset tiles (no warm-matrix DMAs): HAM ramp starts
   at program entry, so the main matmuls run at 2.4GHz.
 - dm/pd DMA first on sync queue; constants packed into 2 DMAs on gpsimd.
 - abs (|f|) for the two hw chunks runs on GpSimd, off the DVE critical path.
 - sinA/cosA are expanded x8 (head dim) on GpSimd and the weights arrive
   host-pre-expanded to [K, 608], so all four U/V-build multiplies are packed
   bf16 tensor_tensor ops (2x DVE mode) instead of stride-0 broadcasts.
 - PSUM in 3 waves (5 pso / 3 psm banks) so chunk-1 matmuls start before
   chunk-0 is evacuated.
 - outputs DMA'd per (m-tile, hw-chunk) as soon as their relu lands, spread
   across sync/gpsimd/tensor queues so enqueues don't serialize.

Sharding: SN x SH = 4 x 2 cores over (N, HW).
"""

import sys

for p in ("/opt/trn_rl_repo", "/root/.axon_site/_ro/trn_rl_repo"):
    if p not in sys.path:
        sys.path.insert(0, p)

import numpy as np
import ml_dtypes
from contextlib import ExitStack

from concourse import bacc, mybir, tile
from concourse.bass_utils import run_bass_kernel_spmd

F32 = mybir.dt.float32
F16 = mybir.dt.float16
BF16 = mybir.dt.bfloat16
I32 = mybir.dt.int32
A = mybir.AluOpType
AF = mybir.ActivationFunctionType

# ---- problem constants ----
N_TOT, H_DM, W_DM = 300, 24, 80
HW_TOT = H_DM * W_DM  # 1920
HEADS = 8
ED = 256
K = ED // 2  # 128
EPS = 1e-5
SCALE = 100.0
TEMPERATURE = 10000.0
TWO_PI = 2.0 * np.pi

# ---- sharding ----
SN, SH = 4, 2
n_per = N_TOT // SN  # 75
n_pad = 76
hw_per = HW_TOT // SH  # 960
M = n_pad * HEADS  # 608
CH = 480  # hw chunk (1 psum bank)
N_WARM = 8
N_FILL = 6

_m_tiles = []
_ms = 0
while _ms < M:
    _m_tiles.append((_ms, min(128, M - _ms)))
    _ms += 128


def _sigma_row():
    k = np.arange(K, dtype=np.float64)
    dim_t = np.float64(TEMPERATURE) ** (k * 2.0 / ED)
    return ((SCALE / dim_t) / TWO_PI).astype(np.float32)  # [128] turns


def _build_program():
    nc = bacc.Bacc("TRN2", target_bir_lowering=False, debug=False)

    pd_d = nc.dram_tensor("pdrow", [1, n_pad], F32, kind="ExternalInput")
    dm_d = nc.dram_tensor("dmrow", [1, 2 * CH], F32, kind="ExternalInput")
    # fp16 hi/lo split of sigma: [:, 0:K]=sig_hi, [:, K:2K]=sig_lo
    sigp_d = nc.dram_tensor("sigp", [1, 2 * K], F16, kind="ExternalInput")
    # [:, :608]=We tiled over n, [:, 608:]=Wo tiled over n
    wp_d = nc.dram_tensor("wpack", [K, 2 * M], BF16, kind="ExternalInput")
    bias_d = nc.dram_tensor("bias_rep", [128, len(_m_tiles)], F32, kind="ExternalInput")
    out_d = nc.dram_tensor("out", [M, hw_per], BF16, kind="ExternalOutput")

    with tile.TileContext(nc) as tc, ExitStack() as ctx:
        sb = ctx.enter_context(tc.tile_pool(name="sb", bufs=1))
        pso = ctx.enter_context(tc.tile_pool(name="pso", bufs=5, space="PSUM"))
        psm = ctx.enter_context(tc.tile_pool(name="psm", bufs=3, space="PSUM"))

        # ---- constants / warm tiles (vector+gpsimd, no DMA deps) ----
        eps_c = sb.tile((128, 1), F32, tag="c_eps")
        nc.gpsimd.memset(eps_c[:], EPS)
        wa = sb.tile((128, 128), BF16, tag="warm_a")
        nc.vector.memset(wa[:], 1.0)
        wb = sb.tile((128, CH), BF16, tag="warm_b")
        nc.vector.memset(wb[:], 1.0)
        twopi_c = sb.tile((128, 1), F32, tag="c_2pi")
        nc.vector.memset(twopi_c[:], TWO_PI)
        negtwopi_c = sb.tile((128, 1), F32, tag="c_n2pi")
        nc.vector.memset(negtwopi_c[:], -TWO_PI)
        halfpi_c = sb.tile((128, 1), F32, tag="c_hpi")
        nc.vector.memset(halfpi_c[:], np.pi / 2)

        # both ACT table loads trigger here, before any data arrives:
        # trig first, then natural_log (whose table must be resident for
        # the Ln ops that follow)
        trigw = sb.tile((1, 1), BF16, tag="trigw")
        nc.scalar.activation(trigw[:], eps_c[0:1], AF.Sin)
        lnw = sb.tile((1, 1), F32, tag="lnw")
        nc.scalar.activation(lnw[:], eps_c[0:1], AF.Ln, bias=eps_c[0:1])

        # ---- input DMAs ----
        dmr = sb.tile((1, 2 * CH), F32, tag="dmr")
        nc.sync.dma_start(dmr[:], dm_d[:])
        pdr = sb.tile((1, n_pad), F32, tag="pdr")
        nc.sync.dma_start(pdr[:], pd_d[:])
        sigp = sb.tile((1, 2 * K), F16, tag="sigp")
        nc.sync.dma_start(sigp[:], sigp_d[:])
        wp = sb.tile((K, 2 * M), BF16, tag="wpack")
        nc.gpsimd.dma_start(wp[:], wp_d[:])
        bias_t = sb.tile((128, len(_m_tiles)), F32, tag="bias")
        nc.gpsimd.dma_start(bias_t[:], bias_d[:])

        # ---- PE warmups from memset tiles: HAM ramp starts now ----
        ps_w = pso.tile((128, CH), F32, tag="pso")
        for _ in range(N_WARM):
            nc.tensor.matmul(ps_w[:], wa[:], wb[:], start=True, stop=True)

        # ---- logs on ACT, per-chunk, with fp16 hi/lo splits ----
        # tau runs as fp16 matmuls (the PE HAM only un-throttles on a
        # sustained 16-bit stream; fp32 matmuls would hold it at 1.2GHz).
        # All rows live on partition 0 (matmul base-partition rule).
        rhs_c = sb.tile((1, 2 * CH), F32, tag="rhs_c")
        LChi = sb.tile((1, 2 * CH), F16, tag="LChi")
        LClo = sb.tile((1, 2 * CH), F16, tag="LClo")
        rhs_a = sb.tile((1, n_pad), F32, tag="rhs_a")
        LAhi = sb.tile((1, n_pad), F16, tag="LAhi")
        LAlo = sb.tile((1, n_pad), F16, tag="LAlo")

        c0, c1 = slice(0, CH), slice(CH, 2 * CH)
        nc.scalar.activation(rhs_c[:, c0], dmr[:, c0], AF.Ln, bias=eps_c[0:1])
        nc.scalar.activation(LChi[:, c0], rhs_c[:, c0], AF.Copy)
        nc.vector.tensor_tensor(LClo[:, c0], rhs_c[:, c0], LChi[:, c0], A.subtract)
        nc.scalar.activation(rhs_c[:, c1], dmr[:, c1], AF.Ln, bias=eps_c[0:1])
        nc.vector.tensor_scalar(pdr[:], pdr[:], 0.0, None, A.max)
        nc.scalar.activation(rhs_a[:], pdr[:], AF.Ln, bias=eps_c[0:1])
        nc.scalar.activation(LAhi[:], rhs_a[:], AF.Copy)
        nc.vector.tensor_tensor(LAlo[:], rhs_a[:], LAhi[:], A.subtract)
        wfill = sb.tile((128, CH), BF16, tag="wfill")
        nc.vector.tensor_copy(wfill[:], wb[:])
        nc.scalar.activation(LChi[:, c1], rhs_c[:, c1], AF.Copy)
        nc.vector.tensor_tensor(LClo[:, c1], rhs_c[:, c1], LChi[:, c1], A.subtract)

        # ---- tau outer products on PE as fp16 hi/lo triples ----
        shi = sigp[0:1, 0:K]
        slo = sigp[0:1, K : 2 * K]
        ps_c0 = psm.tile((K, CH), F32, tag="psm")
        nc.tensor.matmul(ps_c0[:], shi, LChi[:, c0], start=True, stop=False)
        nc.tensor.matmul(ps_c0[:], shi, LClo[:, c0], start=False, stop=False)
        nc.tensor.matmul(ps_c0[:], slo, LChi[:, c0], start=False, stop=True)
        ps_a = psm.tile((K, CH), F32, tag="psm")
        nc.tensor.matmul(ps_a[:, 0:n_pad], shi, LAhi[:], start=True, stop=False)
        nc.tensor.matmul(ps_a[:, 0:n_pad], shi, LAlo[:], start=False, stop=False)
        nc.tensor.matmul(ps_a[:, 0:n_pad], slo, LAhi[:], start=False, stop=True)
        ps_c1 = psm.tile((K, CH), F32, tag="psm")
        nc.tensor.matmul(ps_c1[:], shi, LChi[:, c1], start=True, stop=False)
        nc.tensor.matmul(ps_c1[:], shi, LClo[:, c1], start=False, stop=False)
        nc.tensor.matmul(ps_c1[:], slo, LChi[:, c1], start=False, stop=True)



        # ---- range reduction: rint+sub on DVE, abs on GpSimd ----
        qC0 = sb.tile((K, CH), I32, tag="qC0")
        fC0 = sb.tile((K, CH), F32, tag="fC0")
        uC0 = sb.tile((K, CH), F32, tag="uC0")
        qC1 = sb.tile((K, CH), I32, tag="qC1")
        fC1 = sb.tile((K, CH), F32, tag="fC1")
        uC1 = sb.tile((K, CH), F32, tag="uC1")
        qA = sb.tile((K, n_pad), I32, tag="qA")
        fA = sb.tile((K, n_pad), F32, tag="fA")
        uA = sb.tile((K, n_pad), F32, tag="uA")

        trigA = sb.tile((K, 2 * n_pad), BF16, tag="trigA")  # [sinA | cosA]
        cs_sin = sb.tile((K, hw_per), BF16, tag="cs_sin")
        cs_cos = sb.tile((K, hw_per), BF16, tag="cs_cos")
        U = sb.tile((K, M), BF16, tag="U")
        V = sb.tile((K, M), BF16, tag="V")
        m1 = sb.tile((K, M), BF16, tag="m1")
        m2 = sb.tile((K, M), BF16, tag="m2")
        m3 = sb.tile((K, M), BF16, tag="m3")
        m4 = sb.tile((K, M), BF16, tag="m4")

        def sin2pi(out_ap, in_ap):
            nc.scalar.activation(out_ap, in_ap, AF.Sin, scale=twopi_c[:])

        def cos2pi(out_ap, in_ap):  # in = |f|
            nc.scalar.activation(
                out_ap, in_ap, AF.Sin, bias=halfpi_c[:], scale=negtwopi_c[:]
            )

        def r3(t_ap):  # [K, 608] -> [K, 8, 76]
            return t_ap.rearrange("p (h n) -> p h n", h=HEADS)

        sA = trigA[:, 0:n_pad].unsqueeze(1).to_broadcast((K, HEADS, n_pad))
        cA = trigA[:, n_pad:].unsqueeze(1).to_broadcast((K, HEADS, n_pad))

        # A-side reduce (DVE) + trig, then the U/V build; the C-chunk
        # reduces interleave around it.  rint for the two big C chunks runs
        # on ACT (Copy to int32 rounds-to-nearest), which is idle there.
        nc.vector.tensor_copy(qA[:], ps_a[:, 0:n_pad])
        nc.vector.tensor_tensor(fA[:], ps_a[:, 0:n_pad], qA[:], A.subtract)
        nc.vector.tensor_scalar(
            uA[:].bitcast(I32), fA[:].bitcast(I32), 0x7FFFFFFF, None, A.bitwise_and
        )
        sin2pi(trigA[:, 0:n_pad], fA[:])
        cos2pi(trigA[:, n_pad:], uA[:])
        nc.scalar.activation(qC0[:], ps_c0[:], AF.Copy)
        nc.scalar.activation(qC1[:], ps_c1[:], AF.Copy)

        # fillers pinned mid-pipeline via wfill (a trivial DVE copy of wb
        # emitted after the lo-splits): they keep the PE stream gapless
        # between tau and the main mms so HAM reaches/stays at 8/8
        for _ in range(N_FILL):
            nc.tensor.matmul(ps_w[:], wa[:], wfill[:], start=True, stop=True)

        # U build (gates the first main matmul group)
        nc.vector.tensor_tensor(r3(m1[:]), sA, r3(wp[:, 0:M]), A.mult)  # s*We
        nc.vector.tensor_tensor(r3(m2[:]), cA, r3(wp[:, M:]), A.mult)   # c*Wo
        nc.vector.tensor_tensor(fC0[:], ps_c0[:], qC0[:], A.subtract)
        nc.vector.tensor_tensor(U[:], m1[:], m2[:], A.add)
        nc.vector.tensor_scalar(
            uC0[:].bitcast(I32), fC0[:].bitcast(I32), 0x7FFFFFFF, None,
            A.bitwise_and
        )
        sin2pi(cs_sin[:, 0:CH], fC0[:])
        cos2pi(cs_cos[:, 0:CH], uC0[:])

        # chunk-1 reduce + V build + chunk-1 trig
        nc.vector.tensor_tensor(fC1[:], ps_c1[:], qC1[:], A.subtract)
        nc.vector.tensor_scalar(
            uC1[:].bitcast(I32), fC1[:].bitcast(I32), 0x7FFFFFFF, None,
            A.bitwise_and
        )
        nc.vector.tensor_tensor(r3(m3[:]), sA, r3(wp[:, M:]), A.mult)   # s*Wo
        nc.vector.tensor_tensor(r3(m4[:]), cA, r3(wp[:, 0:M]), A.mult)  # c*We
        nc.vector.tensor_tensor(V[:], m3[:], m4[:], A.subtract)
        sin2pi(cs_sin[:, CH:], fC1[:])
        cos2pi(cs_cos[:, CH:], uC1[:])

        # ---- main matmuls + bias/relu + per-chunk store ----
        # chunk-1 psum tiles: 3 from psm (freed after range reduce), 2 from pso
        relu_on_act = {(0, 0), (0, 2), (0, 4), (1, 1), (1, 3)}
        dma_eng = {
            (0, 0): nc.sync, (0, 1): nc.gpsimd, (0, 2): nc.sync,
            (0, 3): nc.gpsimd, (0, 4): nc.sync,
            (1, 0): nc.scalar, (1, 1): nc.gpsimd, (1, 2): nc.scalar,
            (1, 3): nc.gpsimd, (1, 4): nc.sync,
        }
        obs = {}
        for ci in range(2):
            sl = slice(ci * CH, (ci + 1) * CH)
            ps_os = []
            for mi, (ms, mr) in enumerate(_m_tiles):
                if ci == 1 and mi < 3:
                    ps_o = psm.tile((128, CH), F32, tag="psm")
                else:
                    ps_o = pso.tile((128, CH), F32, tag="pso")
                ps_os.append(ps_o)
                nc.tensor.matmul(
                    ps_o[:mr, :], U[:, ms : ms + mr], cs_cos[:, sl],
                    start=True, stop=False,
                )
            for mi, (ms, mr) in enumerate(_m_tiles):
                ps_o = ps_os[mi]
                nc.tensor.matmul(
                    ps_o[:mr, :], V[:, ms : ms + mr], cs_sin[:, sl],
                    start=False, stop=True,
                )
                if ci == 0:
                    ob_new = sb.tile((128, hw_per), BF16, tag=f"ob{mi}")
                    obs[mi] = ob_new
                ob = obs[mi]
                if (ci, mi) in relu_on_act:
                    nc.scalar.activation(
                        ob[:mr, sl], ps_o[:mr, :], AF.Relu,
                        bias=bias_t[0:mr, mi : mi + 1],
                    )
                else:
                    nc.vector.tensor_scalar(
                        ob[:mr, sl], ps_o[:mr, :], bias_t[0:mr, mi : mi + 1],
                        0.0, A.add, A.max,
                    )
                ms_, mr_ = _m_tiles[mi]
                dma_eng[(ci, mi)].dma_start(
                    out_d[ms_ : ms_ + mr_, sl], ob[:mr_, sl]
                )

    nc.finalize()
    return nc


_NC = None


def _get_nc():
    global _NC
    if _NC is None:
        _NC = _build_program()
    return _NC


def _make_in_maps(predict_depth, depth_map, W, b):
    pd = np.asarray(predict_depth, np.float32).reshape(N_TOT)
    dm = np.asarray(depth_map, np.float32).reshape(HW_TOT)
    W = np.asarray(W, np.float32)
    b = np.asarray(b, np.float32)

    we = W[0::2, :]  # [K, HEADS]
    wo = W[1::2, :]
    # (h,n)-major: col m = h*n_pad + n holds We[:, h]
    wef = np.repeat(we, n_pad, axis=1).astype(ml_dtypes.bfloat16)  # [K, M]
    wof = np.repeat(wo, n_pad, axis=1).astype(ml_dtypes.bfloat16)
    wpack = np.ascontiguousarray(np.concatenate([wef, wof], axis=1))  # [K, 2M]
    # bias per output row m = h*n_pad + n -> b[h]; one [128] column per m-tile
    bias_full = np.zeros(len(_m_tiles) * 128, np.float32)
    bias_full[:M] = np.repeat(b, n_pad)
    bias_rep = np.ascontiguousarray(bias_full.reshape(len(_m_tiles), 128).T)

    k = np.arange(K, dtype=np.float64)
    dim_t = np.float64(TEMPERATURE) ** (k * 2.0 / ED)
    sig64 = (SCALE / dim_t) / TWO_PI  # turns
    sig_hi = sig64.astype(np.float16)
    sig_lo = (sig64 - sig_hi.astype(np.float64)).astype(np.float16)
    sigp = np.zeros((1, 2 * K), np.float16)
    sigp[0, 0:K] = sig_hi
    sigp[0, K : 2 * K] = sig_lo

    in_maps = []
    for c in range(SN * SH):
        ni, hi = c // SH, c % SH
        pd_row = np.zeros((1, n_pad), np.float32)
        pd_row[0, :n_per] = pd[ni * n_per : (ni + 1) * n_per]
        dm_row = np.ascontiguousarray(
            dm[hi * hw_per : (hi + 1) * hw_per].reshape(1, 2 * CH)
        )
        in_maps.append(
            {
                "pdrow": pd_row,
                "dmrow": dm_row,
                "sigp": sigp,
                "wpack": wpack,
                "bias_rep": bias_rep,
            }
        )
    return in_maps


def _run(inputs, trace=False):
    nc = _get_nc()
    in_maps = _make_in_maps(**inputs)
    res = run_bass_kernel_spmd(nc, in_maps, core_ids=list(range(SN * SH)), trace=trace)
    out = np.empty((HEADS, N_TOT, HW_TOT), np.float32)
    for c in range(SN * SH):
        ni, hi = c // SH, c % SH
        blk = (
            np.asarray(res.results[c]["out"])
            .astype(np.float32)
            .reshape(HEADS, n_pad, hw_per)
        )
        n0 = ni * n_per
        out[:, n0 : n0 + n_per, hi * hw_per : (hi + 1) * hw_per] = blk[:, :n_per, :]
    return out, res


def kernel(predict_depth, depth_map, W, b):
    out, _ = _run(
        {"predict_depth": predict_depth, "depth_map": depth_map, "W": W, "b": b}
    )
    return out


# revision 18
# speedup vs baseline: 1.0180x; 1.0180x over previous
"""DepthRelationEmbedding Trainium2 kernel (v4).

Math: out[h,n,hw] = relu( sum_d pos[n,hw,d] * W[d,h] + b[h] ) where pos is the
interleaved sin/cos embedding of delta[n,hw] = ln((relu(pd[n])+eps)/(dm[hw]+eps)).

Angle addition (We = W[0::2], Wo = W[1::2]):
  out[(n,h), hw] = sum_k U[k,(n,h)]*cosC[k,hw] + V[k,(n,h)]*sinC[k,hw]
  U = sinA*We + cosA*Wo,  V = sinA*Wo - cosA*We
so the (N,HW,256) intermediate never exists; per core the output is one
[608 x 256] @ [256 x 960] bf16 matmul pair accumulated in PSUM.

Angles in turns (tau = angle/2pi); range reduction f = tau - rint(tau) via the
fp32->int32 copy (rounds-to-nearest on HW); sin = Sin(2pi f),
cos = Sin(pi/2 - 2pi|f|).

v4 changes vs v3:
 - PE warmups source from memset tiles (no warm-matrix DMAs): HAM ramp starts
   at program entry, so the main matmuls run at 2.4GHz.
 - dm/pd DMA first on sync queue; constants packed into 2 DMAs on gpsimd.
 - abs (|f|) for the two hw chunks runs on GpSimd, off the DVE critical path.
 - sinA/cosA are expanded x8 (head dim) on GpSimd and the weights arrive
   host-pre-expanded to [K, 608], so all four U/V-build multiplies are packed
   bf16 tensor_tensor ops (2x DVE mode) instead of stride-0 broadcasts.
 - PSUM in 3 waves (5 pso / 3 psm banks) so chunk-1 matmuls start before
   chunk-0 is evacuated.
 - outputs DMA'd per (m-tile, hw-chunk) as soon as their relu lands, spread
   across sync/gpsimd/tensor queues so enqueues don't serialize.

Sharding: SN x SH = 4 x 2 cores over (N, HW).
"""

import sys

for p in ("/opt/trn_rl_repo", "/root/.axon_site/_ro/trn_rl_repo"):
    if p not in sys.path:
        sys.path.insert(0, p)

import numpy as np
import ml_dtypes
from contextlib import ExitStack

from concourse import bacc, mybir, tile
from concourse.bass_utils import run_bass_kernel_spmd

F32 = mybir.dt.float32
F16 = mybir.dt.float16
BF16 = mybir.dt.bfloat16
I32 = mybir.dt.int32
A = mybir.AluOpType
AF = mybir.ActivationFunctionType

# ---- problem constants ----
N_TOT, H_DM, W_DM = 300, 24, 80
HW_TOT = H_DM * W_DM  # 1920
HEADS = 8
ED = 256
K = ED // 2  # 128
EPS = 1e-5
SCALE = 100.0
TEMPERATURE = 10000.0
TWO_PI = 2.0 * np.pi

# ---- sharding ----
SN, SH = 4, 2
n_per = N_TOT // SN  # 75
n_pad = 76
hw_per = HW_TOT // SH  # 960
M = n_pad * HEADS  # 608
CH = 480  # hw chunk (1 psum bank)
N_WARM = 8
N_FILL = 13

_m_tiles = []
_ms = 0
while _ms < M:
    _m_tiles.append((_ms, min(128, M - _ms)))
    _ms += 128


def _sigma_row():
    k = np.arange(K, dtype=np.float64)
    dim_t = np.float64(TEMPERATURE) ** (k * 2.0 / ED)
    return ((SCALE / dim_t) / TWO_PI).astype(np.float32)  # [128] turns


def _build_program():
    nc = bacc.Bacc("TRN2", target_bir_lowering=False, debug=False)

    pd_d = nc.dram_tensor("pdrow", [1, n_pad], F32, kind="ExternalInput")
    dm_d = nc.dram_tensor("dmrow", [1, 2 * CH], F32, kind="ExternalInput")
    # fp16 hi/lo split of sigma: [:, 0:K]=sig_hi, [:, K:2K]=sig_lo
    sigp_d = nc.dram_tensor("sigp", [1, 2 * K], F16, kind="ExternalInput")
    # [:, :608]=We tiled over n, [:, 608:]=Wo tiled over n
    wp_d = nc.dram_tensor("wpack", [K, 2 * M], BF16, kind="ExternalInput")
    bias_d = nc.dram_tensor("bias_rep", [128, len(_m_tiles)], F32, kind="ExternalInput")
    out_d = nc.dram_tensor("out", [M, hw_per], BF16, kind="ExternalOutput")

    with tile.TileContext(nc) as tc, ExitStack() as ctx:
        sb = ctx.enter_context(tc.tile_pool(name="sb", bufs=1))
        pso = ctx.enter_context(tc.tile_pool(name="pso", bufs=5, space="PSUM"))
        psm = ctx.enter_context(tc.tile_pool(name="psm", bufs=3, space="PSUM"))

        # ---- constants / warm tiles (vector+gpsimd, no DMA deps) ----
        eps_c = sb.tile((128, 1), F32, tag="c_eps")
        nc.gpsimd.memset(eps_c[:], EPS)
        wa = sb.tile((128, 128), BF16, tag="warm_a")
        nc.vector.memset(wa[:], 1.0)
        wb = sb.tile((128, CH), BF16, tag="warm_b")
        nc.vector.memset(wb[:], 1.0)
        twopi_c = sb.tile((128, 1), F32, tag="c_2pi")
        nc.vector.memset(twopi_c[:], TWO_PI)
        negtwopi_c = sb.tile((128, 1), F32, tag="c_n2pi")
        nc.vector.memset(negtwopi_c[:], -TWO_PI)
        halfpi_c = sb.tile((128, 1), F32, tag="c_hpi")
        nc.vector.memset(halfpi_c[:], np.pi / 2)

        # natural_log table load triggers here, before any data arrives
        lnw = sb.tile((1, 1), F32, tag="lnw")
        nc.scalar.activation(lnw[:], eps_c[0:1], AF.Ln, bias=eps_c[0:1])

        # ---- input DMAs ----
        dmr = sb.tile((1, 2 * CH), F32, tag="dmr")
        nc.sync.dma_start(dmr[:], dm_d[:])
        pdr = sb.tile((1, n_pad), F32, tag="pdr")
        nc.sync.dma_start(pdr[:], pd_d[:])
        sigp = sb.tile((1, 2 * K), F16, tag="sigp")
        nc.sync.dma_start(sigp[:], sigp_d[:])
        wp = sb.tile((K, 2 * M), BF16, tag="wpack")
        nc.gpsimd.dma_start(wp[:], wp_d[:])
        bias_t = sb.tile((128, len(_m_tiles)), F32, tag="bias")
        nc.gpsimd.dma_start(bias_t[:], bias_d[:])

        # ---- PE warmups from memset tiles: HAM ramp starts now ----
        ps_w = pso.tile((128, CH), F32, tag="pso")
        for _ in range(N_WARM):
            nc.tensor.matmul(ps_w[:], wa[:], wb[:], start=True, stop=True)

        # ---- logs on ACT, per-chunk, with fp16 hi/lo splits ----
        # tau runs as fp16 matmuls (the PE HAM only un-throttles on a
        # sustained 16-bit stream; fp32 matmuls would hold it at 1.2GHz).
        # All rows live on partition 0 (matmul base-partition rule).
        rhs_c = sb.tile((1, 2 * CH), F32, tag="rhs_c")
        LChi = sb.tile((1, 2 * CH), F16, tag="LChi")
        LClo = sb.tile((1, 2 * CH), F16, tag="LClo")
        rhs_a = sb.tile((1, n_pad), F32, tag="rhs_a")
        LAhi = sb.tile((1, n_pad), F16, tag="LAhi")
        LAlo = sb.tile((1, n_pad), F16, tag="LAlo")

        c0, c1 = slice(0, CH), slice(CH, 2 * CH)
        nc.scalar.activation(rhs_c[:, c0], dmr[:, c0], AF.Ln, bias=eps_c[0:1])
        nc.scalar.activation(LChi[:, c0], rhs_c[:, c0], AF.Copy)
        nc.vector.tensor_tensor(LClo[:, c0], rhs_c[:, c0], LChi[:, c0], A.subtract)
        nc.scalar.activation(rhs_c[:, c1], dmr[:, c1], AF.Ln, bias=eps_c[0:1])
        nc.vector.tensor_scalar(pdr[:], pdr[:], 0.0, None, A.max)
        nc.scalar.activation(rhs_a[:], pdr[:], AF.Ln, bias=eps_c[0:1])
        nc.scalar.activation(LAhi[:], rhs_a[:], AF.Copy)
        nc.vector.tensor_tensor(LAlo[:], rhs_a[:], LAhi[:], A.subtract)
        wfill = sb.tile((128, CH), BF16, tag="wfill")
        nc.vector.tensor_copy(wfill[:], wb[:])

        nc.scalar.activation(LChi[:, c1], rhs_c[:, c1], AF.Copy)
        nc.vector.tensor_tensor(LClo[:, c1], rhs_c[:, c1], LChi[:, c1], A.subtract)

        # ---- tau outer products on PE as fp16 hi/lo triples ----
        shi = sigp[0:1, 0:K]
        slo = sigp[0:1, K : 2 * K]
        ps_c0 = psm.tile((K, CH), F32, tag="psm")
        nc.tensor.matmul(ps_c0[:], shi, LChi[:, c0], start=True, stop=False)
        nc.tensor.matmul(ps_c0[:], shi, LClo[:, c0], start=False, stop=False)
        nc.tensor.matmul(ps_c0[:], slo, LChi[:, c0], start=False, stop=True)
        ps_a = psm.tile((K, CH), F32, tag="psm")
        nc.tensor.matmul(ps_a[:, 0:n_pad], shi, LAhi[:], start=True, stop=False)
        nc.tensor.matmul(ps_a[:, 0:n_pad], shi, LAlo[:], start=False, stop=False)
        nc.tensor.matmul(ps_a[:, 0:n_pad], slo, LAhi[:], start=False, stop=True)
        ps_c1 = psm.tile((K, CH), F32, tag="psm")
        nc.tensor.matmul(ps_c1[:], shi, LChi[:, c1], start=True, stop=False)
        nc.tensor.matmul(ps_c1[:], shi, LClo[:, c1], start=False, stop=False)
        nc.tensor.matmul(ps_c1[:], slo, LChi[:, c1], start=False, stop=True)

        # chunk-0 rint on ACT before the trig table load (Copy runs under
        # the ln set), then the trig load triggers, hidden behind tau
        qC0 = sb.tile((K, CH), I32, tag="qC0")
        nc.scalar.activation(qC0[:], ps_c0[:], AF.Copy)
        trigw = sb.tile((1, 1), BF16, tag="trigw")
        nc.scalar.activation(trigw[:], rhs_a[0:1, 0:1], AF.Sin)



        # ---- range reduction: rint+sub on DVE ----
        fC0 = sb.tile((K, CH), F32, tag="fC0")
        uC0 = sb.tile((K, CH), F32, tag="uC0")
        qC1 = sb.tile((K, CH), I32, tag="qC1")
        fC1 = sb.tile((K, CH), F32, tag="fC1")
        uC1 = sb.tile((K, CH), F32, tag="uC1")
        qA = sb.tile((K, n_pad), I32, tag="qA")
        fA = sb.tile((K, n_pad), F32, tag="fA")
        uA = sb.tile((K, n_pad), F32, tag="uA")

        trigA = sb.tile((K, 2 * n_pad), BF16, tag="trigA")  # [sinA | cosA]
        cs_sin = sb.tile((K, hw_per), BF16, tag="cs_sin")
        cs_cos = sb.tile((K, hw_per), BF16, tag="cs_cos")
        U = sb.tile((K, M), BF16, tag="U")
        V = sb.tile((K, M), BF16, tag="V")
        m1 = sb.tile((K, M), BF16, tag="m1")
        m2 = sb.tile((K, M), BF16, tag="m2")
        m3 = sb.tile((K, M), BF16, tag="m3")
        m4 = sb.tile((K, M), BF16, tag="m4")

        def sin2pi(out_ap, in_ap):
            nc.scalar.activation(out_ap, in_ap, AF.Sin, scale=twopi_c[:])

        def cos2pi(out_ap, in_ap):  # in = |f|
            nc.scalar.activation(
                out_ap, in_ap, AF.Sin, bias=halfpi_c[:], scale=negtwopi_c[:]
            )

        def r3(t_ap):  # [K, 608] -> [K, 8, 76]
            return t_ap.rearrange("p (h n) -> p h n", h=HEADS)

        sA = trigA[:, 0:n_pad].unsqueeze(1).to_broadcast((K, HEADS, n_pad))
        cA = trigA[:, n_pad:].unsqueeze(1).to_broadcast((K, HEADS, n_pad))

        # A-side reduce (DVE) + trig, then the U/V build; the C-chunk
        # reduces interleave around it.  rint for the two big C chunks runs
        # on ACT (Copy to int32 rounds-to-nearest), which is idle there.
        nc.vector.tensor_copy(qA[:], ps_a[:, 0:n_pad])
        nc.vector.tensor_tensor(fA[:], ps_a[:, 0:n_pad], qA[:], A.subtract)
        nc.vector.tensor_scalar(
            uA[:].bitcast(I32), fA[:].bitcast(I32), 0x7FFFFFFF, None, A.bitwise_and
        )
        sin2pi(trigA[:, 0:n_pad], fA[:])
        cos2pi(trigA[:, n_pad:], uA[:])
        nc.scalar.activation(qC1[:], ps_c1[:], AF.Copy)

        # fillers pinned mid-pipeline via wfill (a trivial DVE copy of wb
        # emitted after the lo-splits): they keep the PE stream gapless
        # between tau and the main mms so HAM reaches/stays at 8/8
        for _ in range(N_FILL):
            nc.tensor.matmul(ps_w[:], wa[:], wfill[:], start=True, stop=True)

        # U build (gates the first main matmul group)
        nc.vector.tensor_tensor(r3(m1[:]), sA, r3(wp[:, 0:M]), A.mult)  # s*We
        nc.vector.tensor_tensor(r3(m2[:]), cA, r3(wp[:, M:]), A.mult)   # c*Wo
        nc.vector.tensor_tensor(fC0[:], ps_c0[:], qC0[:], A.subtract)
        nc.vector.tensor_tensor(U[:], m1[:], m2[:], A.add)
        nc.vector.tensor_scalar(
            uC0[:].bitcast(I32), fC0[:].bitcast(I32), 0x7FFFFFFF, None,
            A.bitwise_and
        )
        sin2pi(cs_sin[:, 0:CH], fC0[:])
        cos2pi(cs_cos[:, 0:CH], uC0[:])

        # chunk-1 reduce + V build + chunk-1 trig
        nc.vector.tensor_tensor(fC1[:], ps_c1[:], qC1[:], A.subtract)
        nc.vector.tensor_scalar(
            uC1[:].bitcast(I32), fC1[:].bitcast(I32), 0x7FFFFFFF, None,
            A.bitwise_and
        )
        nc.vector.tensor_tensor(r3(m3[:]), sA, r3(wp[:, M:]), A.mult)   # s*Wo
        nc.vector.tensor_tensor(r3(m4[:]), cA, r3(wp[:, 0:M]), A.mult)  # c*We
        nc.vector.tensor_tensor(V[:], m3[:], m4[:], A.subtract)
        sin2pi(cs_sin[:, CH:], fC1[:])
        cos2pi(cs_cos[:, CH:], uC1[:])

        # ---- main matmuls + bias/relu + per-chunk store ----
        # chunk-1 psum tiles: 3 from psm (freed after range reduce), 2 from pso
        relu_on_act = {(0, 0), (0, 2), (0, 4), (1, 1), (1, 3)}
        dma_eng = {
            (0, 0): nc.sync, (0, 1): nc.gpsimd, (0, 2): nc.sync,
            (0, 3): nc.gpsimd, (0, 4): nc.sync,
            (1, 0): nc.scalar, (1, 1): nc.gpsimd, (1, 2): nc.scalar,
            (1, 3): nc.gpsimd, (1, 4): nc.sync,
        }
        obs = {}
        for ci in range(2):
            sl = slice(ci * CH, (ci + 1) * CH)
            ps_os = []
            for mi, (ms, mr) in enumerate(_m_tiles):
                if ci == 1 and mi < 3:
                    ps_o = psm.tile((128, CH), F32, tag="psm")
                else:
                    ps_o = pso.tile((128, CH), F32, tag="pso")
                ps_os.append(ps_o)
                nc.tensor.matmul(
                    ps_o[:mr, :], U[:, ms : ms + mr], cs_cos[:, sl],
                    start=True, stop=False,
                )
            for mi, (ms, mr) in enumerate(_m_tiles):
                ps_o = ps_os[mi]
                nc.tensor.matmul(
                    ps_o[:mr, :], V[:, ms : ms + mr], cs_sin[:, sl],
                    start=False, stop=True,
                )
                if ci == 0:
                    ob_new = sb.tile((128, hw_per), BF16, tag=f"ob{mi}")
                    obs[mi] = ob_new
                ob = obs[mi]
                if (ci, mi) in relu_on_act:
                    nc.scalar.activation(
                        ob[:mr, sl], ps_o[:mr, :], AF.Relu,
                        bias=bias_t[0:mr, mi : mi + 1],
                    )
                else:
                    nc.vector.tensor_scalar(
                        ob[:mr, sl], ps_o[:mr, :], bias_t[0:mr, mi : mi + 1],
                        0.0, A.add, A.max,
                    )
                ms_, mr_ = _m_tiles[mi]
                dma_eng[(ci, mi)].dma_start(
                    out_d[ms_ : ms_ + mr_, sl], ob[:mr_, sl]
                )

    nc.finalize()
    return nc


_NC = None


def _get_nc():
    global _NC
    if _NC is None:
        _NC = _build_program()
    return _NC


def _make_in_maps(predict_depth, depth_map, W, b):
    pd = np.asarray(predict_depth, np.float32).reshape(N_TOT)
    dm = np.asarray(depth_map, np.float32).reshape(HW_TOT)
    W = np.asarray(W, np.float32)
    b = np.asarray(b, np.float32)

    we = W[0::2, :]  # [K, HEADS]
    wo = W[1::2, :]
    # (h,n)-major: col m = h*n_pad + n holds We[:, h]
    wef = np.repeat(we, n_pad, axis=1).astype(ml_dtypes.bfloat16)  # [K, M]
    wof = np.repeat(wo, n_pad, axis=1).astype(ml_dtypes.bfloat16)
    wpack = np.ascontiguousarray(np.concatenate([wef, wof], axis=1))  # [K, 2M]
    # bias per output row m = h*n_pad + n -> b[h]; one [128] column per m-tile
    bias_full = np.zeros(len(_m_tiles) * 128, np.float32)
    bias_full[:M] = np.repeat(b, n_pad)
    bias_rep = np.ascontiguousarray(bias_full.reshape(len(_m_tiles), 128).T)

    k = np.arange(K, dtype=np.float64)
    dim_t = np.float64(TEMPERATURE) ** (k * 2.0 / ED)
    sig64 = (SCALE / dim_t) / TWO_PI  # turns
    sig_hi = sig64.astype(np.float16)
    sig_lo = (sig64 - sig_hi.astype(np.float64)).astype(np.float16)
    sigp = np.zeros((1, 2 * K), np.float16)
    sigp[0, 0:K] = sig_hi
    sigp[0, K : 2 * K] = sig_lo

    in_maps = []
    for c in range(SN * SH):
        ni, hi = c // SH, c % SH
        pd_row = np.zeros((1, n_pad), np.float32)
        pd_row[0, :n_per] = pd[ni * n_per : (ni + 1) * n_per]
        dm_row = np.ascontiguousarray(
            dm[hi * hw_per : (hi + 1) * hw_per].reshape(1, 2 * CH)
        )
        in_maps.append(
            {
                "pdrow": pd_row,
                "dmrow": dm_row,
                "sigp": sigp,
                "wpack": wpack,
                "bias_rep": bias_rep,
            }
        )
    return in_maps


def _run(inputs, trace=False):
    nc = _get_nc()
    in_maps = _make_in_maps(**inputs)
    res = run_bass_kernel_spmd(nc, in_maps, core_ids=list(range(SN * SH)), trace=trace)
    out = np.empty((HEADS, N_TOT, HW_TOT), np.float32)
    for c in range(SN * SH):
        ni, hi = c // SH, c % SH
        blk = (
            np.asarray(res.results[c]["out"])
            .astype(np.float32)
            .reshape(HEADS, n_pad, hw_per)
        )
        n0 = ni * n_per
        out[:, n0 : n0 + n_per, hi * hw_per : (hi + 1) * hw_per] = blk[:, :n_per, :]
    return out, res


def kernel(predict_depth, depth_map, W, b):
    out, _ = _run(
        {"predict_depth": predict_depth, "depth_map": depth_map, "W": W, "b": b}
    )
    return out


# revision 20
# speedup vs baseline: 1.0718x; 1.0528x over previous
"""DepthRelationEmbedding Trainium2 kernel (v4).

Math: out[h,n,hw] = relu( sum_d pos[n,hw,d] * W[d,h] + b[h] ) where pos is the
interleaved sin/cos embedding of delta[n,hw] = ln((relu(pd[n])+eps)/(dm[hw]+eps)).

Angle addition (We = W[0::2], Wo = W[1::2]):
  out[(n,h), hw] = sum_k U[k,(n,h)]*cosC[k,hw] + V[k,(n,h)]*sinC[k,hw]
  U = sinA*We + cosA*Wo,  V = sinA*Wo - cosA*We
so the (N,HW,256) intermediate never exists; per core the output is one
[608 x 256] @ [256 x 960] bf16 matmul pair accumulated in PSUM.

Angles in turns (tau = angle/2pi); range reduction f = tau - rint(tau) via the
fp32->int32 copy (rounds-to-nearest on HW); sin = Sin(2pi f),
cos = Sin(pi/2 - 2pi|f|).

v4 changes vs v3:
 - PE warmups source from memset tiles (no warm-matrix DMAs): HAM ramp starts
   at program entry, so the main matmuls run at 2.4GHz.
 - dm/pd DMA first on sync queue; constants packed into 2 DMAs on gpsimd.
 - abs (|f|) for the two hw chunks runs on GpSimd, off the DVE critical path.
 - sinA/cosA are expanded x8 (head dim) on GpSimd and the weights arrive
   host-pre-expanded to [K, 608], so all four U/V-build multiplies are packed
   bf16 tensor_tensor ops (2x DVE mode) instead of stride-0 broadcasts.
 - PSUM in 3 waves (5 pso / 3 psm banks) so chunk-1 matmuls start before
   chunk-0 is evacuated.
 - outputs DMA'd per (m-tile, hw-chunk) as soon as their relu lands, spread
   across sync/gpsimd/tensor queues so enqueues don't serialize.

Sharding: SN x SH = 4 x 2 cores over (N, HW).
"""

import sys

for p in ("/opt/trn_rl_repo", "/root/.axon_site/_ro/trn_rl_repo"):
    if p not in sys.path:
        sys.path.insert(0, p)

import numpy as np
import ml_dtypes
from contextlib import ExitStack

from concourse import bacc, mybir, tile
from concourse.bass_utils import run_bass_kernel_spmd

F32 = mybir.dt.float32
F16 = mybir.dt.float16
BF16 = mybir.dt.bfloat16
I32 = mybir.dt.int32
A = mybir.AluOpType
AF = mybir.ActivationFunctionType

# ---- problem constants ----
N_TOT, H_DM, W_DM = 300, 24, 80
HW_TOT = H_DM * W_DM  # 1920
HEADS = 8
ED = 256
K = ED // 2  # 128
EPS = 1e-5
SCALE = 100.0
TEMPERATURE = 10000.0
TWO_PI = 2.0 * np.pi

# ---- sharding ----
SN, SH = 4, 2
n_per = N_TOT // SN  # 75
n_pad = 76
hw_per = HW_TOT // SH  # 960
M = n_pad * HEADS  # 608
CH = 480  # hw chunk (1 psum bank)
N_WARM = 8
N_FILL = 13

_m_tiles = []
_ms = 0
while _ms < M:
    _m_tiles.append((_ms, min(128, M - _ms)))
    _ms += 128


def _sigma_row():
    k = np.arange(K, dtype=np.float64)
    dim_t = np.float64(TEMPERATURE) ** (k * 2.0 / ED)
    return ((SCALE / dim_t) / TWO_PI).astype(np.float32)  # [128] turns


def _build_program():
    nc = bacc.Bacc("TRN2", target_bir_lowering=False, debug=False)

    pd_d = nc.dram_tensor("pdrow", [1, n_pad], F32, kind="ExternalInput")
    dm_d = nc.dram_tensor("dmrow", [1, 2 * CH], F32, kind="ExternalInput")
    # fp16 hi/lo split of sigma: [:, 0:K]=sig_hi, [:, K:2K]=sig_lo
    sigp_d = nc.dram_tensor("sigp", [1, 2 * K], F16, kind="ExternalInput")
    # [:, :608]=We tiled over n, [:, 608:]=Wo tiled over n
    wp_d = nc.dram_tensor("wpack", [K, 2 * M], BF16, kind="ExternalInput")
    bias_d = nc.dram_tensor("bias_rep", [128, len(_m_tiles)], F32, kind="ExternalInput")
    out_d = nc.dram_tensor("out", [M, hw_per], BF16, kind="ExternalOutput")

    with tile.TileContext(nc) as tc, ExitStack() as ctx:
        sb = ctx.enter_context(tc.tile_pool(name="sb", bufs=1))
        pso = ctx.enter_context(tc.tile_pool(name="pso", bufs=5, space="PSUM"))
        psm = ctx.enter_context(tc.tile_pool(name="psm", bufs=3, space="PSUM"))

        # ---- constants / warm tiles (vector+gpsimd, no DMA deps) ----
        eps_c = sb.tile((128, 1), F32, tag="c_eps")
        nc.gpsimd.memset(eps_c[:], EPS)
        wa = sb.tile((128, 128), BF16, tag="warm_a")
        nc.vector.memset(wa[:], 1.0)
        wb = sb.tile((128, CH), BF16, tag="warm_b")
        nc.vector.memset(wb[:], 1.0)
        twopi_c = sb.tile((128, 1), F32, tag="c_2pi")
        nc.vector.memset(twopi_c[:], TWO_PI)
        negtwopi_c = sb.tile((128, 1), F32, tag="c_n2pi")
        nc.vector.memset(negtwopi_c[:], -TWO_PI)
        halfpi_c = sb.tile((128, 1), F32, tag="c_hpi")
        nc.vector.memset(halfpi_c[:], np.pi / 2)

        # natural_log table load triggers here, before any data arrives
        lnw = sb.tile((1, 1), F32, tag="lnw")
        nc.scalar.activation(lnw[:], eps_c[0:1], AF.Ln, bias=eps_c[0:1])

        # ---- input DMAs ----
        dmr = sb.tile((1, 2 * CH), F32, tag="dmr")
        nc.sync.dma_start(dmr[:], dm_d[:])
        pdr = sb.tile((1, n_pad), F32, tag="pdr")
        nc.sync.dma_start(pdr[:], pd_d[:])
        sigp = sb.tile((1, 2 * K), F16, tag="sigp")
        nc.sync.dma_start(sigp[:], sigp_d[:])
        wp = sb.tile((K, 2 * M), BF16, tag="wpack")
        nc.gpsimd.dma_start(wp[:], wp_d[:])
        bias_t = sb.tile((128, len(_m_tiles)), F32, tag="bias")
        nc.gpsimd.dma_start(bias_t[:], bias_d[:])

        # ---- PE warmups from memset tiles: HAM ramp starts now ----
        ps_w = pso.tile((128, CH), F32, tag="pso")
        for _ in range(N_WARM):
            nc.tensor.matmul(ps_w[:], wa[:], wb[:], start=True, stop=True)

        # ---- logs on ACT, per-chunk, with fp16 hi/lo splits ----
        # tau runs as fp16 matmuls (the PE HAM only un-throttles on a
        # sustained 16-bit stream; fp32 matmuls would hold it at 1.2GHz).
        # All rows live on partition 0 (matmul base-partition rule).
        rhs_c = sb.tile((1, 2 * CH), F32, tag="rhs_c")
        LChi = sb.tile((1, 2 * CH), F16, tag="LChi")
        LClo = sb.tile((1, 2 * CH), F16, tag="LClo")
        rhs_a = sb.tile((1, n_pad), F32, tag="rhs_a")
        LAhi = sb.tile((1, n_pad), F16, tag="LAhi")
        LAlo = sb.tile((1, n_pad), F16, tag="LAlo")

        c0, c1 = slice(0, CH), slice(CH, 2 * CH)
        nc.scalar.activation(rhs_c[:, c0], dmr[:, c0], AF.Ln, bias=eps_c[0:1])
        nc.scalar.activation(rhs_c[:, c1], dmr[:, c1], AF.Ln, bias=eps_c[0:1])
        nc.vector.tensor_scalar(pdr[:], pdr[:], 0.0, None, A.max)
        nc.scalar.activation(rhs_a[:], pdr[:], AF.Ln, bias=eps_c[0:1])
        nc.scalar.activation(LAhi[:], rhs_a[:], AF.Copy)
        # the fp16 hi/lo splits of the dm logs run on DVE (idle early);
        # LAhi stays on ACT (tiny, and ACT is free right after Ln(pd))
        nc.vector.tensor_copy(LChi[:, c0], rhs_c[:, c0])
        nc.vector.tensor_tensor(LClo[:, c0], rhs_c[:, c0], LChi[:, c0], A.subtract)
        nc.vector.tensor_copy(LChi[:, c1], rhs_c[:, c1])
        nc.vector.tensor_tensor(LClo[:, c1], rhs_c[:, c1], LChi[:, c1], A.subtract)
        nc.vector.tensor_tensor(LAlo[:], rhs_a[:], LAhi[:], A.subtract)
        wfill = sb.tile((128, CH), BF16, tag="wfill")
        nc.vector.tensor_copy(wfill[:], wb[:])

        # ---- tau outer products on PE as fp16 hi/lo triples ----
        # within each chunk the two hi-rhs products go first (ready before
        # the lo split lands); the accumulate with stop=True goes last
        shi = sigp[0:1, 0:K]
        slo = sigp[0:1, K : 2 * K]
        ps_c0 = psm.tile((K, CH), F32, tag="psm")
        nc.tensor.matmul(ps_c0[:], shi, LChi[:, c0], start=True, stop=False)
        nc.tensor.matmul(ps_c0[:], slo, LChi[:, c0], start=False, stop=False)
        nc.tensor.matmul(ps_c0[:], shi, LClo[:, c0], start=False, stop=True)
        ps_a = psm.tile((K, CH), F32, tag="psm")
        nc.tensor.matmul(ps_a[:, 0:n_pad], shi, LAhi[:], start=True, stop=False)
        nc.tensor.matmul(ps_a[:, 0:n_pad], slo, LAhi[:], start=False, stop=False)
        nc.tensor.matmul(ps_a[:, 0:n_pad], shi, LAlo[:], start=False, stop=True)
        ps_c1 = psm.tile((K, CH), F32, tag="psm")
        nc.tensor.matmul(ps_c1[:], shi, LChi[:, c1], start=True, stop=False)
        nc.tensor.matmul(ps_c1[:], slo, LChi[:, c1], start=False, stop=False)
        nc.tensor.matmul(ps_c1[:], shi, LClo[:, c1], start=False, stop=True)

        # chunk-0 rint on ACT before the trig table load (Copy runs under
        # the ln set), then the trig load triggers, hidden behind tau
        qC0 = sb.tile((K, CH), I32, tag="qC0")
        nc.scalar.activation(qC0[:], ps_c0[:], AF.Copy)
        trigw = sb.tile((1, 1), BF16, tag="trigw")
        nc.scalar.activation(trigw[:], rhs_a[0:1, 0:1], AF.Sin)



        # ---- range reduction: rint+sub on DVE ----
        fC0 = sb.tile((K, CH), F32, tag="fC0")
        uC0 = sb.tile((K, CH), F32, tag="uC0")
        qC1 = sb.tile((K, CH), I32, tag="qC1")
        fC1 = sb.tile((K, CH), F32, tag="fC1")
        uC1 = sb.tile((K, CH), F32, tag="uC1")
        qA = sb.tile((K, n_pad), I32, tag="qA")
        fA = sb.tile((K, n_pad), F32, tag="fA")
        uA = sb.tile((K, n_pad), F32, tag="uA")

        trigA = sb.tile((K, 2 * n_pad), BF16, tag="trigA")  # [sinA | cosA]
        cs_sin = sb.tile((K, hw_per), BF16, tag="cs_sin")
        cs_cos = sb.tile((K, hw_per), BF16, tag="cs_cos")
        U = sb.tile((K, M), BF16, tag="U")
        V = sb.tile((K, M), BF16, tag="V")
        m1 = sb.tile((K, M), BF16, tag="m1")
        m2 = sb.tile((K, M), BF16, tag="m2")
        m3 = sb.tile((K, M), BF16, tag="m3")
        m4 = sb.tile((K, M), BF16, tag="m4")

        def sin2pi(out_ap, in_ap):
            nc.scalar.activation(out_ap, in_ap, AF.Sin, scale=twopi_c[:])

        def cos2pi(out_ap, in_ap):  # in = |f|
            nc.scalar.activation(
                out_ap, in_ap, AF.Sin, bias=halfpi_c[:], scale=negtwopi_c[:]
            )

        def r3(t_ap):  # [K, 608] -> [K, 8, 76]
            return t_ap.rearrange("p (h n) -> p h n", h=HEADS)

        sA = trigA[:, 0:n_pad].unsqueeze(1).to_broadcast((K, HEADS, n_pad))
        cA = trigA[:, n_pad:].unsqueeze(1).to_broadcast((K, HEADS, n_pad))

        # A-side reduce (DVE) + trig, then the U/V build; the C-chunk
        # reduces interleave around it.  rint for the two big C chunks runs
        # on ACT (Copy to int32 rounds-to-nearest), which is idle there.
        nc.vector.tensor_copy(qA[:], ps_a[:, 0:n_pad])
        nc.vector.tensor_tensor(fA[:], ps_a[:, 0:n_pad], qA[:], A.subtract)
        nc.vector.tensor_scalar(
            uA[:].bitcast(I32), fA[:].bitcast(I32), 0x7FFFFFFF, None, A.bitwise_and
        )
        sin2pi(trigA[:, 0:n_pad], fA[:])
        cos2pi(trigA[:, n_pad:], uA[:])
        nc.scalar.activation(qC1[:], ps_c1[:], AF.Copy)

        # fillers pinned mid-pipeline via wfill (a trivial DVE copy of wb
        # emitted after the lo-splits): they keep the PE stream gapless
        # between tau and the main mms so HAM reaches/stays at 8/8
        for _ in range(N_FILL):
            nc.tensor.matmul(ps_w[:], wa[:], wfill[:], start=True, stop=True)

        # chunk-0 reduce; cos first (it gates the U-side mains), then the
        # U build
        nc.vector.tensor_tensor(fC0[:], ps_c0[:], qC0[:], A.subtract)
        nc.vector.tensor_scalar(
            uC0[:].bitcast(I32), fC0[:].bitcast(I32), 0x7FFFFFFF, None,
            A.bitwise_and
        )
        cos2pi(cs_cos[:, 0:CH], uC0[:])
        sin2pi(cs_sin[:, 0:CH], fC0[:])
        nc.vector.tensor_tensor(r3(m1[:]), sA, r3(wp[:, 0:M]), A.mult)  # s*We
        nc.vector.tensor_tensor(r3(m2[:]), cA, r3(wp[:, M:]), A.mult)   # c*Wo
        nc.vector.tensor_tensor(U[:], m1[:], m2[:], A.add)

        # chunk-1 reduce + V build + chunk-1 trig
        nc.vector.tensor_tensor(fC1[:], ps_c1[:], qC1[:], A.subtract)
        nc.vector.tensor_scalar(
            uC1[:].bitcast(I32), fC1[:].bitcast(I32), 0x7FFFFFFF, None,
            A.bitwise_and
        )
        nc.vector.tensor_tensor(r3(m3[:]), sA, r3(wp[:, M:]), A.mult)   # s*Wo
        nc.vector.tensor_tensor(r3(m4[:]), cA, r3(wp[:, 0:M]), A.mult)  # c*We
        nc.vector.tensor_tensor(V[:], m3[:], m4[:], A.subtract)
        sin2pi(cs_sin[:, CH:], fC1[:])
        cos2pi(cs_cos[:, CH:], uC1[:])

        # ---- main matmuls + bias/relu + per-chunk store ----
        # chunk-1 psum tiles: 3 from psm (freed after range reduce), 2 from pso
        relu_on_act = {(0, 0), (0, 2), (0, 4), (1, 1), (1, 3)}
        dma_eng = {
            (0, 0): nc.sync, (0, 1): nc.gpsimd, (0, 2): nc.sync,
            (0, 3): nc.gpsimd, (0, 4): nc.sync,
            (1, 0): nc.scalar, (1, 1): nc.gpsimd, (1, 2): nc.scalar,
            (1, 3): nc.gpsimd, (1, 4): nc.sync,
        }
        obs = {}
        for ci in range(2):
            sl = slice(ci * CH, (ci + 1) * CH)
            ps_os = []
            for mi, (ms, mr) in enumerate(_m_tiles):
                if ci == 1 and mi < 3:
                    ps_o = psm.tile((128, CH), F32, tag="psm")
                else:
                    ps_o = pso.tile((128, CH), F32, tag="pso")
                ps_os.append(ps_o)
                nc.tensor.matmul(
                    ps_o[:mr, :], U[:, ms : ms + mr], cs_cos[:, sl],
                    start=True, stop=False,
                )
            for mi, (ms, mr) in enumerate(_m_tiles):
                ps_o = ps_os[mi]
                nc.tensor.matmul(
                    ps_o[:mr, :], V[:, ms : ms + mr], cs_sin[:, sl],
                    start=False, stop=True,
                )
                if ci == 0:
                    ob_new = sb.tile((128, hw_per), BF16, tag=f"ob{mi}")
                    obs[mi] = ob_new
                ob = obs[mi]
                if (ci, mi) in relu_on_act:
                    nc.scalar.activation(
                        ob[:mr, sl], ps_o[:mr, :], AF.Relu,
                        bias=bias_t[0:mr, mi : mi + 1],
                    )
                else:
                    nc.vector.tensor_scalar(
                        ob[:mr, sl], ps_o[:mr, :], bias_t[0:mr, mi : mi + 1],
                        0.0, A.add, A.max,
                    )
                ms_, mr_ = _m_tiles[mi]
                dma_eng[(ci, mi)].dma_start(
                    out_d[ms_ : ms_ + mr_, sl], ob[:mr_, sl]
                )

    nc.finalize()
    return nc


_NC = None


def _get_nc():
    global _NC
    if _NC is None:
        _NC = _build_program()
    return _NC


def _make_in_maps(predict_depth, depth_map, W, b):
    pd = np.asarray(predict_depth, np.float32).reshape(N_TOT)
    dm = np.asarray(depth_map, np.float32).reshape(HW_TOT)
    W = np.asarray(W, np.float32)
    b = np.asarray(b, np.float32)

    we = W[0::2, :]  # [K, HEADS]
    wo = W[1::2, :]
    # (h,n)-major: col m = h*n_pad + n holds We[:, h]
    wef = np.repeat(we, n_pad, axis=1).astype(ml_dtypes.bfloat16)  # [K, M]
    wof = np.repeat(wo, n_pad, axis=1).astype(ml_dtypes.bfloat16)
    wpack = np.ascontiguousarray(np.concatenate([wef, wof], axis=1))  # [K, 2M]
    # bias per output row m = h*n_pad + n -> b[h]; one [128] column per m-tile
    bias_full = np.zeros(len(_m_tiles) * 128, np.float32)
    bias_full[:M] = np.repeat(b, n_pad)
    bias_rep = np.ascontiguousarray(bias_full.reshape(len(_m_tiles), 128).T)

    k = np.arange(K, dtype=np.float64)
    dim_t = np.float64(TEMPERATURE) ** (k * 2.0 / ED)
    sig64 = (SCALE / dim_t) / TWO_PI  # turns
    sig_hi = sig64.astype(np.float16)
    sig_lo = (sig64 - sig_hi.astype(np.float64)).astype(np.float16)
    sigp = np.zeros((1, 2 * K), np.float16)
    sigp[0, 0:K] = sig_hi
    sigp[0, K : 2 * K] = sig_lo

    in_maps = []
    for c in range(SN * SH):
        ni, hi = c // SH, c % SH
        pd_row = np.zeros((1, n_pad), np.float32)
        pd_row[0, :n_per] = pd[ni * n_per : (ni + 1) * n_per]
        dm_row = np.ascontiguousarray(
            dm[hi * hw_per : (hi + 1) * hw_per].reshape(1, 2 * CH)
        )
        in_maps.append(
            {
                "pdrow": pd_row,
                "dmrow": dm_row,
                "sigp": sigp,
                "wpack": wpack,
                "bias_rep": bias_rep,
            }
        )
    return in_maps


def _run(inputs, trace=False):
    nc = _get_nc()
    in_maps = _make_in_maps(**inputs)
    res = run_bass_kernel_spmd(nc, in_maps, core_ids=list(range(SN * SH)), trace=trace)
    out = np.empty((HEADS, N_TOT, HW_TOT), np.float32)
    for c in range(SN * SH):
        ni, hi = c // SH, c % SH
        blk = (
            np.asarray(res.results[c]["out"])
            .astype(np.float32)
            .reshape(HEADS, n_pad, hw_per)
        )
        n0 = ni * n_per
        out[:, n0 : n0 + n_per, hi * hw_per : (hi + 1) * hw_per] = blk[:, :n_per, :]
    return out, res


def kernel(predict_depth, depth_map, W, b):
    out, _ = _run(
        {"predict_depth": predict_depth, "depth_map": depth_map, "W": W, "b": b}
    )
    return out
